# revision 53
# baseline (speedup 1.0000x reference)
"""Trainium2 Bass kernel for the 1x1-conv attention block + groupnorm-swish.

Reference computation (B=2, C=128, spatial 16^3 -> N=4096):
    q = wq@query + bq; k = wk@key + bk; v = wv@value + bv   (per batch, [C, N])
    S[i, j] = sum_c q[c,i] k[c,j]; P = softmax_j(S)
    h[c, i] = sum_j v[c,j] P[i,j]
    x = wo@h + bo + value
    out = silu(group_norm(x) * gamma + beta)   (G=32 groups of 4 channels)

Sharding: 8 cores = 2 batches x 4 query-token chunks of 1024 (sequence
parallel). One tiny AllReduce per batch-group of 4 cores produces the
group-norm statistics.

Structure (vs the v1 baseline at 127us):
- BOTH q and k projections fold into a single query-side projection:
  S = (Wq xq + bq)^T (Wk xk)  [bk dropped: constant over keys, softmax-
  invariant]  =  xk^T (M2 xq + c0)  with M2 = Wk^T Wq, c0 = Wk^T bq both
  host-precomputed (weight-only transforms). The S^T matmul stationary is
  the RAW bf16 key tile straight from DMA - no k-projection, no PSUM
  eviction on the k path at all.
- v arrives host-TRANSPOSED ([j, t, c] tiles, a pure layout permutation)
  in bf16 and is upcast to fp32r on the vector engine's loop slack, so
  the PV stationary is raw v^T. The fused output projection W2 = wo@wv
  applies once post-loop (one matmul), and the residual (value + wo@bv +
  bo) is added via an identity-stationary matmul accumulating into the
  same PSUM, leaving x in PSUM for bn_stats and the final silu to read
  in place.
- Main loop: S^T in pure bf16 (raw key stationary x bf16 q2), exp out in
  fp32r (the scalar engine is dtype-independent and bf16 out is SLOWER),
  PV/denominator in fp32r. Per tile the PE runs ~1.05us vs the scalar
  exp floor of ~1.11us. Denominator columns [0:128] accumulate on the
  tensor engine (ones-stationary PSUM accumulation lands broadcast
  across partitions); columns [128:1024] on the vector engine.
- 1/denominator uses the DVE reciprocal_approx_fast custom op (~51 ULP)
  - no Ln/Exp epilogue passes and no activation-table switches.
- Group rstd = 1/sqrt(var+eps) via the magic-constant Newton iteration
  on the DVE (one iteration; its sign flip is folded into host-negated
  gamma, and the 1/16 group averaging into the host e matrix).
- Tail: the groupnorm stats AllReduce (4-core groups) has a one-time
  ~11us cold CC init, so a warm AllReduce gated on an early-loop v^T
  chunk runs it concurrently with the loop; the real AllReduce then
  inits in ~1us and pays only mesh events + inter-core skew. The silu
  table load hides under the loop. (A remote_dma P2P stats exchange was
  prototyped but fabric semaphore arrival takes ~ms on this axon setup.)
"""

import sys
import types

import ml_dtypes
import numpy as np

# The axon NTFF-profile hook module is absent from this image's antenv
# package; concourse imports it unconditionally when tracing. Install a
# functional shim (used by the test harness; harmless otherwise).
try:
    import antenv.axon_hooks  # noqa: F401
except ImportError:
    import antenv

    _mod = types.ModuleType("antenv.axon_hooks")
    _hook_box = [None]
    _mod.set_axon_ntff_profile_hook = lambda h: _hook_box.__setitem__(0, h)
    _mod.get_axon_ntff_profile_hook = lambda: _hook_box[0]
    sys.modules["antenv.axon_hooks"] = _mod
    antenv.axon_hooks = _mod
    try:
        from trn_agent_boot.trn_boot import _ntff_profile_via_ctypes

        _mod.set_axon_ntff_profile_hook(
            _ntff_profile_via_ctypes("/opt/axon/libaxon_pjrt.so")
        )
    except Exception:
        pass

import concourse.tile as tile
from concourse import bacc, mybir
from concourse.bass_utils import run_bass_kernel_spmd

B = 2
C = 128
N = 4096
NCORES = 8
CHUNKS = 4  # query-token chunks per batch
NC = N // CHUNKS  # 1024 tokens per core
JT = N // 128  # 32 key tiles of 128
G = 32  # groupnorm groups
EPS = 1e-5
DB_PE = 128  # denominator columns summed on the tensor engine

F32 = mybir.dt.float32
I32 = mybir.dt.int32
R = mybir.dt.float32r
BF16 = mybir.dt.bfloat16
AF = mybir.ActivationFunctionType
ALU = mybir.AluOpType

_NC_CACHE = None


def _build():
    nc = bacc.Bacc("TRN2", target_bir_lowering=False, debug=False, num_devices=NCORES)


    q_in = nc.dram_tensor("q_in", [C, NC], BF16, kind="ExternalInput")
    k_in = nc.dram_tensor("k_in", [C, N], BF16, kind="ExternalInput")
    vt_in = nc.dram_tensor("vt_in", [128, JT * C], BF16, kind="ExternalInput")
    vres_in = nc.dram_tensor("vres_in", [C, NC], BF16, kind="ExternalInput")
    # packed weights: [m2T | w2T | eye | ones] bf16 and
    # [ones_f32 | e | c0 | bo_eff | gamma | beta] f32, et [G, C] f32.
    wb_in = nc.dram_tensor("wb", [C, 4 * C], BF16, kind="ExternalInput")
    wf_in = nc.dram_tensor("wf", [C, C + G + 6], F32, kind="ExternalInput")
    et_in = nc.dram_tensor("et", [G, C], F32, kind="ExternalInput")
    y_out = nc.dram_tensor("y_out", [C, NC], F32, kind="ExternalOutput")

    with tile.TileContext(nc) as tc:
        with (
            tc.tile_pool(name="const", bufs=1) as const,
            tc.tile_pool(name="big", bufs=1) as big,
            tc.tile_pool(name="expp", bufs=3) as expp,
            tc.tile_pool(name="psum", bufs=2, space="PSUM") as psum,
            tc.tile_pool(name="dram", bufs=2, space="DRAM") as dram,
        ):
            wb = const.tile([C, 4 * C], BF16)
            wf = const.tile([C, C + G + 6], F32)
            et_sb = const.tile([G, C], F32)
            q_raw = big.tile([C, NC], BF16)

            nc.sync.dma_start(wb[:], wb_in[:])
            nc.sync.dma_start(wf[:], wf_in[:])
            nc.sync.dma_start(q_raw[:, 0:512], q_in[:, 0:512])
            nc.sync.dma_start(q_raw[:, 512:1024], q_in[:, 512:1024])
            nc.sync.dma_start(et_sb[:], et_in[:])

            m2T = wb[:, 0:C]
            w2T = wb[:, C : 2 * C]
            eye = wb[:, 2 * C : 3 * C]
            ones_b = wb[:, 3 * C : 4 * C]
            ones_f = wf[:, 0:C]
            e_sb = wf[:, C : C + G]
            c0_sb = wf[:, C + G : C + G + 1]
            boe_sb = wf[:, C + G + 1 : C + G + 2]
            gamma_sb = wf[:, C + G + 2 : C + G + 3]
            beta_sb = wf[:, C + G + 3 : C + G + 4]
            # int32 bit-pattern constants for the magic-Newton rsqrt
            sh1_i = wf[0:G, C + G + 4 : C + G + 5].bitcast(I32)
            magic_i = wf[0:G, C + G + 5 : C + G + 6].bitcast(I32)

            # k / v^T / v-residual inputs stream behind the q path, ordered
            # by first use: alternating k / v^T chunks of 8 tiles each.
            k_raw = big.tile([C, N], BF16)
            vt_raw = big.tile([128, JT, C], BF16)
            vt_sb = big.tile([128, JT, C], R)
            vres = big.tile([C, NC], BF16)
            ones_r = const.tile([C, DB_PE], R)
            nc.vector.memset(ones_r[:].bitcast(F32), 1.0)
            vt_flat = vt_raw[:].rearrange("j t c -> j (t c)")
            nc.sync.dma_start(k_raw[:, 0:256], k_in[:, 0:256])
            nc.sync.dma_start(vt_flat[:, 0:256], vt_in[:, 0:256])
            nc.sync.dma_start(k_raw[:, 256:1024], k_in[:, 256:1024])
            nc.sync.dma_start(vt_flat[:, 256:1024], vt_in[:, 256:1024])
            for h in range(1, 4):
                qs = slice(h * (N // 4), (h + 1) * (N // 4))
                nc.sync.dma_start(k_raw[:, qs], k_in[:, qs])
                nc.sync.dma_start(vt_flat[:, qs], vt_in[:, qs])
            nc.scalar.dma_start(vres[:], vres_in[:])

            # ---- single folded projection: q2 = M2 @ xq + c0 ----
            q2_ps = psum.tile([C, NC], F32, tag="st", name="q2ps")
            for h in range(2):
                sl = slice(h * 512, (h + 1) * 512)
                nc.tensor.matmul(q2_ps[:, sl], m2T, q_raw[:, sl], start=True, stop=True)
            q2_sb = big.tile([C, NC], BF16)
            for h in range(2):
                sl = slice(h * 512, (h + 1) * 512)
                nc.vector.tensor_scalar(
                    out=q2_sb[:, sl], in0=q2_ps[:, sl], scalar1=c0_sb,
                    scalar2=None, op0=ALU.add,
                )

            # residual r = v_chunk + bo_eff (bf16; feeds the identity matmul)
            r_sb = big.tile([C, NC], BF16)
            nc.vector.tensor_scalar(
                out=r_sb[:], in0=vres[:], scalar1=boe_sb, scalar2=None, op0=ALU.add
            )

            # ---- main attention loop over 32 key tiles ----
            # per tile: S^T = k_tile^T @ q2 (psum) -> exp (ACT, ->sbuf bf16)
            #           h  += v^T_tile @ exp     (PSUM accumulate)
            #           db += ones    @ exp      (PSUM accumulate = denominator)
            #           acc += exp[:, 256:]      (vector f32 accumulate)
            k3 = k_raw[:].rearrange("c (t j) -> c t j", j=128)
            h_ps = psum.tile([C, NC], F32, tag="h", bufs=1)
            db_ps = psum.tile([C, DB_PE], F32, tag="b1")
            acc_sb = big.tile([128, NC - DB_PE], F32)

            def qk(t, st):
                for h in range(2):
                    sl = slice(h * 512, (h + 1) * 512)
                    nc.tensor.matmul(
                        st[:, sl], k3[:, t, :], q2_sb[:, sl], start=True, stop=True
                    )

            # upcast the first v^T chunk before the loop; the rest interleave
            # with the loop on the vector engine's slack (~8 tiles ahead).
            vt3f = vt_sb[:].rearrange("j t c -> j (t c)")
            nc.vector.tensor_copy(vt3f[:, 0 : 4 * C], vt_flat[:, 0 : 4 * C])

            # Warm AllReduce gated on an early-loop v^T upcast chunk: its
            # one-time ~11us cold CC init runs while the loop computes, so
            # the real stats AllReduce below starts on a warm CC path.
            ccw_in = dram.tile([G, 1], F32, name="ccw_in")
            ccw_out = dram.tile([G, 1], F32, name="ccw_out")
            nc.scalar.dma_start(ccw_in[:], vt_sb[0:G, 8, 0:1].bitcast(F32))
            nc.gpsimd.collective_compute(
                "AllReduce",
                ALU.add,
                replica_groups=[[0, 1, 2, 3], [4, 5, 6, 7]],
                ins=[ccw_in.opt()],
                outs=[ccw_out.opt()],
            )

            st_tiles = {}
            st_tiles[0] = psum.tile([128, NC], F32, tag="st", name="st0")
            qk(0, st_tiles[0])
            for t in range(JT):
                if t % 4 == 0 and t + 4 < JT:
                    cs = slice((t + 4) * C, min(t + 8, JT) * C)
                    nc.vector.tensor_copy(vt3f[:, cs], vt_flat[:, cs])
                if t + 1 < JT:
                    st_tiles[t + 1] = psum.tile(
                        [128, NC], F32, tag="st", name=f"st{t + 1}"
                    )
                    qk(t + 1, st_tiles[t + 1])
                exp_t = expp.tile([128, NC], R, tag="exp")
                nc.scalar.activation(out=exp_t[:], in_=st_tiles.pop(t)[:], func=AF.Exp)
                # PV first after exp; the ones stationary load for db then
                # shadows under the second PV matmul.
                for h in range(2):
                    sl = slice(h * 512, (h + 1) * 512)
                    nc.tensor.matmul(
                        h_ps[:, sl], vt_sb[:, t, :], exp_t[:, sl],
                        start=(t == 0), stop=(t == JT - 1), skip_group_check=True,
                    )
                nc.tensor.matmul(
                    db_ps[:], ones_r[:, 0:C], exp_t[:, 0:DB_PE],
                    start=(t == 0), stop=(t == JT - 1), skip_group_check=True,
                )
                if t == 0:
                    nc.vector.tensor_copy(acc_sb[:], exp_t[:, DB_PE:NC].bitcast(F32))
                else:
                    nc.vector.tensor_add(
                        acc_sb[:], acc_sb[:], exp_t[:, DB_PE:NC].bitcast(F32)
                    )

            # ---- denominator -> 1/db broadcast across partitions ----
            # ones_f32 matmul sums acc_sb over partitions AND broadcasts.
            db2_ps = psum.tile([C, NC - DB_PE], F32, tag="st")
            half = (NC - DB_PE) // 2
            for h in range(2):
                sl = slice(h * half, (h + 1) * half)
                nc.tensor.matmul(
                    db2_ps[:, sl], ones_f, acc_sb[:, sl], start=True, stop=True
                )
            dinv_sb = big.tile([C, NC], F32)
            nc.vector.reciprocal_approx_fast(dinv_sb[:, 0:DB_PE], db_ps[:])
            nc.vector.reciprocal_approx_fast(dinv_sb[:, DB_PE:NC], db2_ps[:])

            # ---- x = W2 @ (vP * dinv) + r  (residual via identity matmul,
            # x stays in PSUM for bn_stats and the final silu); per-half
            # pipelined: half 0's matmuls and bn_stats overlap half 1's mul.
            tmp_sb = big.tile([C, NC], BF16)
            x_ps = psum.tile([C, NC], F32, tag="st", name="xps")
            bstats = big.tile([C, 2, nc.vector.BN_STATS_DIM], F32)
            for h in range(2):
                sl = slice(h * 512, (h + 1) * 512)
                nc.vector.tensor_mul(tmp_sb[:, sl], h_ps[:, sl], dinv_sb[:, sl])
                nc.tensor.matmul(
                    x_ps[:, sl], w2T, tmp_sb[:, sl],
                    start=True, stop=False, skip_group_check=True,
                )
                nc.tensor.matmul(
                    x_ps[:, sl], eye, r_sb[:, sl],
                    start=False, stop=True, skip_group_check=True,
                )

            # ---- groupnorm partial stats: per-channel [mean, E[x^2]] ----
            for hh in range(2):
                nc.vector.bn_stats(
                    out=bstats[:, hh, :], in_=x_ps[:, hh * 512 : (hh + 1) * 512]
                )
            mv = big.tile([C, nc.vector.BN_AGGR_DIM], F32)
            nc.vector.bn_aggr(out=mv[:], in_=bstats[:])
            rowstats = big.tile([C, 2], F32)
            nc.vector.tensor_copy(rowstats[:, 0:1], mv[:, 0:1])
            nc.vector.tensor_mul(rowstats[:, 1:2], mv[:, 0:1], mv[:, 0:1])
            nc.vector.tensor_add(rowstats[:, 1:2], rowstats[:, 1:2], mv[:, 1:2])
            # force the silu table load NOW so it overlaps the AllReduce
            warm_sb = const.tile([G, 1], F32)
            nc.scalar.activation(out=warm_sb[:], in_=wf[0:G, 0:1], func=AF.Silu)

            # ---- AllReduce partial stats within each batch's 4-core group ----
            gs_ps = psum.tile([G, 2], F32, tag="b1", name="gsps")
            nc.tensor.matmul(gs_ps[:], e_sb, rowstats[:], start=True, stop=True)
            gs_sb = big.tile([G, 2], F32)
            nc.vector.tensor_copy(gs_sb[:], gs_ps[:])
            cc_in = dram.tile([G, 2], F32)
            cc_out = dram.tile([G, 2], F32)
            nc.scalar.dma_start(cc_in[:], gs_sb[:])
            nc.gpsimd.collective_compute(
                "AllReduce",
                ALU.add,
                replica_groups=[[0, 1, 2, 3], [4, 5, 6, 7]],
                ins=[cc_in.opt()],
                outs=[cc_out.opt()],
            )
            own = big.tile([G, 2], F32)
            nc.sync.dma_start(own[:], cc_out[:])

            # ---- group mean / rstd -> per-channel scale+bias ----
            # own = [mean_g, E[x^2]_g] (the 1/16 group scaling is folded into
            # the host-provided e matrix).
            msr = own  # [mean, rstd] in place
            varx = big.tile([G, 1], F32)
            nc.vector.tensor_mul(varx[:], msr[:, 0:1], msr[:, 0:1])
            # var + eps = (E[x^2] + eps) - mean^2, fused
            nc.vector.scalar_tensor_tensor(
                out=varx[:], in0=msr[:, 1:2], scalar=EPS, in1=varx[:],
                op0=ALU.add, op1=ALU.subtract,
            )
            # magic-constant Newton rsqrt on [G,1] (no sqrt table). ONE
            # iteration: result is -rsqrt (sign restored via host-negated
            # gamma); |err| ~0.2% which the 2e-2 budget absorbs.
            vh = big.tile([G, 1], F32)
            nc.vector.tensor_scalar(
                out=vh[:], in0=varx[:], scalar1=0.5, scalar2=None, op0=ALU.mult
            )
            yi = big.tile([G, 1], F32)
            yint = yi[:].bitcast(I32)
            nc.vector.tensor_scalar(
                out=yint, in0=varx[:].bitcast(I32), scalar1=sh1_i, scalar2=None,
                op0=ALU.logical_shift_right,
            )
            nc.vector.tensor_sub(yint, magic_i, yint)
            t1 = big.tile([G, 1], F32)
            nc.vector.tensor_mul(t1[:], yi[:], yi[:])
            nc.vector.tensor_mul(t1[:], t1[:], vh[:])
            nc.vector.tensor_scalar(
                out=t1[:], in0=t1[:], scalar1=1.5, scalar2=None, op0=ALU.subtract
            )
            nc.vector.tensor_mul(yi[:], t1[:], yi[:])
            nc.vector.tensor_copy(msr[:, 1:2], yi[:])

            exp_ps = psum.tile([C, 2], F32, tag="b1", name="expps")
            nc.tensor.matmul(exp_ps[:], et_sb[:], msr[:], start=True, stop=True)
            mr_sb = big.tile([C, 2], F32)
            nc.vector.tensor_copy(mr_sb[:], exp_ps[:])
            fs_sb = big.tile([C, 1], F32)
            nc.vector.tensor_mul(fs_sb[:], mr_sb[:, 1:2], gamma_sb)
            fb_sb = big.tile([C, 1], F32)
            nc.vector.tensor_mul(fb_sb[:], mr_sb[:, 0:1], fs_sb[:])
            nc.vector.tensor_sub(fb_sb[:], beta_sb, fb_sb[:])

            # ---- out = silu(fs * x + fb), store overlapping the 2nd half ----
            y_sb = big.tile([C, NC], F32)
            for hh in range(2):
                sl = slice(hh * 512, (hh + 1) * 512)
                nc.scalar.activation(
                    out=y_sb[:, sl], in_=x_ps[:, sl], func=AF.Silu,
                    bias=fb_sb[:], scale=fs_sb[:],
                )
                nc.sync.dma_start(y_out[:, sl], y_sb[:, sl])

    nc.compile()
    return nc


def _get_nc():
    global _NC_CACHE
    if _NC_CACHE is None:
        _NC_CACHE = _build()
    return _NC_CACHE


def _in_maps(query, key, value, wq, bq, wk, bk, wv, bv, wo, bo, gamma, beta):
    f32 = lambda a: np.ascontiguousarray(np.asarray(a, dtype=np.float32))
    bf16 = lambda a: np.ascontiguousarray(np.asarray(a).astype(ml_dtypes.bfloat16))
    q = f32(query).reshape(B, C, N)
    k = f32(key).reshape(B, C, N)
    v = f32(value).reshape(B, C, N)
    wq, wk, wv, wo = f32(wq), f32(wk), f32(wv), f32(wo)
    bo_eff = (wo @ f32(bv).reshape(C) + f32(bo).reshape(C)).astype(np.float32)
    c0 = (wk.T @ f32(bq).reshape(C)).astype(np.float32)

    m2T = wq.T @ wk  # lhsT for q2 = (Wk^T Wq) @ xq
    w2T = (wo @ wv).T  # lhsT for x = W2 @ (vP * dinv)
    eye = np.eye(C, dtype=np.float32)
    ones = np.ones((C, C), dtype=np.float32)
    wb = bf16(np.concatenate([m2T, w2T, eye, ones], axis=1))
    e = np.zeros((C, G), dtype=np.float32)
    e[np.arange(C), np.arange(C) // (C // G)] = 1.0
    icons = np.broadcast_to(
        np.array([1, 0x5F3759DF], np.int32).view(np.float32), (C, 2)
    )
    # 1/16 group averaging folded into e; gamma negated to absorb the
    # odd-iteration sign of the on-device Newton rsqrt.
    wf = np.concatenate(
        [ones, e / 16.0,
         c0[:, None], bo_eff[:, None],
         -f32(gamma).reshape(C, 1), f32(beta).reshape(C, 1), icons], axis=1
    ).astype(np.float32)
    et = np.ascontiguousarray(e.T)  # [G, C]
    shared = {"wb": wb, "wf": np.ascontiguousarray(wf), "et": et}

    maps = []
    for p in range(NCORES):
        b, ch = divmod(p, CHUNKS)
        sl = slice(ch * NC, (ch + 1) * NC)
        # rotate the key/value token axis so this core's chunk sits at j=0;
        # attention is permutation-invariant over keys, and the residual
        # slice becomes vres at the same offset on every core.
        rot = np.roll(np.arange(N), -ch * NC)
        kr = k[b][:, rot]
        vr = v[b][:, rot]
        # v^T tiles [j, t, c]: vt[j, t*C + c] = vr[c, t*128 + j]
        vt = np.ascontiguousarray(
            vr.reshape(C, JT, 128).transpose(2, 1, 0).reshape(128, JT * C)
        )
        maps.append(
            {
                "q_in": bf16(q[b][:, sl]),
                "k_in": bf16(kr),
                "vt_in": bf16(vt),
                "vres_in": bf16(vr[:, 0:NC]),
                **shared,
            }
        )
    return maps


def kernel(query, key, value, wq, bq, wk, bk, wv, bv, wo, bo, gamma, beta):
    nc = _get_nc()
    maps = _in_maps(query, key, value, wq, bq, wk, bk, wv, bv, wo, bo, gamma, beta)
    res = run_bass_kernel_spmd(nc, maps, list(range(NCORES)))
    out = np.empty((B, C, N), dtype=np.float32)
    for p in range(NCORES):
        b, ch = divmod(p, CHUNKS)
        out[b][:, ch * NC : (ch + 1) * NC] = res.results[p]["y_out"]
    return out.reshape(B, C, 16, 16, 16)


# revision 54
# speedup vs baseline: 1.1205x; 1.1205x over previous
"""Trainium2 Bass kernel for the 1x1-conv attention block + groupnorm-swish.

Reference computation (B=2, C=128, spatial 16^3 -> N=4096):
    q = wq@query + bq; k = wk@key + bk; v = wv@value + bv   (per batch, [C, N])
    S[i, j] = sum_c q[c,i] k[c,j]; P = softmax_j(S)
    h[c, i] = sum_j v[c,j] P[i,j]
    x = wo@h + bo + value
    out = silu(group_norm(x) * gamma + beta)   (G=32 groups of 4 channels)

Sharding: 8 cores = 2 batches x 4 query-token chunks of 1024 (sequence
parallel). One tiny AllReduce per batch-group of 4 cores produces the
group-norm statistics.

Structure (vs the v1 baseline at 127us):
- BOTH q and k projections fold into a single query-side projection:
  S = (Wq xq + bq)^T (Wk xk)  [bk dropped: constant over keys, softmax-
  invariant]  =  xk^T (M2 xq + c0)  with M2 = Wk^T Wq, c0 = Wk^T bq both
  host-precomputed (weight-only transforms). The S^T matmul stationary is
  the RAW bf16 key tile straight from DMA - no k-projection, no PSUM
  eviction on the k path at all.
- v arrives host-TRANSPOSED ([j, t, c] tiles, a pure layout permutation)
  in bf16 and is upcast to fp32r on the vector engine's loop slack, so
  the PV stationary is raw v^T. The fused output projection W2 = wo@wv
  applies once post-loop (one matmul), and the residual (value + wo@bv +
  bo) is added via an identity-stationary matmul accumulating into the
  same PSUM, leaving x in PSUM for bn_stats and the final silu to read
  in place.
- Main loop: S^T in pure bf16 (raw key stationary x bf16 q2), exp out in
  fp32r (the scalar engine is dtype-independent and bf16 out is SLOWER),
  PV/denominator in fp32r. Per tile the PE runs ~1.05us vs the scalar
  exp floor of ~1.11us. Denominator columns [0:128] accumulate on the
  tensor engine (ones-stationary PSUM accumulation lands broadcast
  across partitions); columns [128:1024] on the vector engine.
- 1/denominator uses the DVE reciprocal_approx_fast custom op (~51 ULP)
  - no Ln/Exp epilogue passes and no activation-table switches.
- Group rstd = 1/sqrt(var+eps) via the magic-constant Newton iteration
  on the DVE (one iteration; its sign flip is folded into host-negated
  gamma, and the 1/16 group averaging into the host e matrix).
- Tail: the groupnorm stats AllReduce (4-core groups) has a one-time
  ~11us cold CC init, so a warm AllReduce gated on an early-loop v^T
  chunk runs it concurrently with the loop; the real AllReduce then
  inits in ~1us and pays only mesh events + inter-core skew. The silu
  table load hides under the loop. (A remote_dma P2P stats exchange was
  prototyped but fabric semaphore arrival takes ~ms on this axon setup.)
"""

import sys
import types

import ml_dtypes
import numpy as np

# The axon NTFF-profile hook module is absent from this image's antenv
# package; concourse imports it unconditionally when tracing. Install a
# functional shim (used by the test harness; harmless otherwise).
try:
    import antenv.axon_hooks  # noqa: F401
except ImportError:
    import antenv

    _mod = types.ModuleType("antenv.axon_hooks")
    _hook_box = [None]
    _mod.set_axon_ntff_profile_hook = lambda h: _hook_box.__setitem__(0, h)
    _mod.get_axon_ntff_profile_hook = lambda: _hook_box[0]
    sys.modules["antenv.axon_hooks"] = _mod
    antenv.axon_hooks = _mod
    try:
        from trn_agent_boot.trn_boot import _ntff_profile_via_ctypes

        _mod.set_axon_ntff_profile_hook(
            _ntff_profile_via_ctypes("/opt/axon/libaxon_pjrt.so")
        )
    except Exception:
        pass

import concourse.tile as tile
from concourse import bacc, mybir
from concourse.bass_utils import run_bass_kernel_spmd

B = 2
C = 128
N = 4096
NCORES = 8
CHUNKS = 4  # query-token chunks per batch
NC = N // CHUNKS  # 1024 tokens per core
JT = N // 128  # 32 key tiles of 128
G = 32  # groupnorm groups
EPS = 1e-5
DB_PE = 128  # denominator columns summed on the tensor engine

F32 = mybir.dt.float32
I32 = mybir.dt.int32
R = mybir.dt.float32r
BF16 = mybir.dt.bfloat16
AF = mybir.ActivationFunctionType
ALU = mybir.AluOpType

_NC_CACHE = None


def _build():
    nc = bacc.Bacc("TRN2", target_bir_lowering=False, debug=False, num_devices=NCORES)


    q_in = nc.dram_tensor("q_in", [C, NC], BF16, kind="ExternalInput")
    k_in = nc.dram_tensor("k_in", [C, N], BF16, kind="ExternalInput")
    vt_in = nc.dram_tensor("vt_in", [128, JT * C], BF16, kind="ExternalInput")
    vres_in = nc.dram_tensor("vres_in", [C, NC], BF16, kind="ExternalInput")
    # packed weights: [m2T | w2T | eye | ones] bf16 and
    # [ones_f32 | e | c0 | bo_eff | gamma | beta] f32, et [G, C] f32.
    wb_in = nc.dram_tensor("wb", [C, 4 * C], BF16, kind="ExternalInput")
    wf_in = nc.dram_tensor("wf", [C, C + G + 6], F32, kind="ExternalInput")
    et_in = nc.dram_tensor("et", [G, C], F32, kind="ExternalInput")
    y_out = nc.dram_tensor("y_out", [C, NC], F32, kind="ExternalOutput")

    with tile.TileContext(nc) as tc:
        with (
            tc.tile_pool(name="const", bufs=1) as const,
            tc.tile_pool(name="big", bufs=1) as big,
            tc.tile_pool(name="expp", bufs=3) as expp,
            tc.tile_pool(name="psum", bufs=2, space="PSUM") as psum,
            tc.tile_pool(name="dram", bufs=2, space="DRAM") as dram,
        ):
            wb = const.tile([C, 4 * C], BF16)
            wf = const.tile([C, C + G + 6], F32)
            et_sb = const.tile([G, C], F32)
            q_raw = big.tile([C, NC], BF16)

            nc.sync.dma_start(wb[:], wb_in[:])
            nc.sync.dma_start(wf[:], wf_in[:])
            nc.sync.dma_start(q_raw[:, 0:512], q_in[:, 0:512])
            nc.sync.dma_start(q_raw[:, 512:1024], q_in[:, 512:1024])
            nc.sync.dma_start(et_sb[:], et_in[:])

            m2T = wb[:, 0:C]
            w2T = wb[:, C : 2 * C]
            eye = wb[:, 2 * C : 3 * C]
            ones_b = wb[:, 3 * C : 4 * C]
            ones_f = wf[:, 0:C]
            e_sb = wf[:, C : C + G]
            c0_sb = wf[:, C + G : C + G + 1]
            boe_sb = wf[:, C + G + 1 : C + G + 2]
            gamma_sb = wf[:, C + G + 2 : C + G + 3]
            beta_sb = wf[:, C + G + 3 : C + G + 4]
            # int32 bit-pattern constants for the magic-Newton rsqrt
            sh1_i = wf[0:G, C + G + 4 : C + G + 5].bitcast(I32)
            magic_i = wf[0:G, C + G + 5 : C + G + 6].bitcast(I32)

            # k / v^T / v-residual inputs stream behind the q path, ordered
            # by first use: alternating k / v^T chunks of 8 tiles each.
            k_raw = big.tile([C, N], BF16)
            vt_raw = big.tile([128, JT, C], BF16)
            vt_sb = big.tile([128, JT, C], R)
            vres = big.tile([C, NC], BF16)
            ones_r = const.tile([C, DB_PE], R)
            nc.vector.memset(ones_r[:].bitcast(F32), 1.0)
            vt_flat = vt_raw[:].rearrange("j t c -> j (t c)")
            for h in range(8):
                qs = slice(h * 512, (h + 1) * 512)
                nc.sync.dma_start(k_raw[:, qs], k_in[:, qs])
                nc.sync.dma_start(vt_flat[:, qs], vt_in[:, qs])

            # ---- single folded projection: q2 = M2 @ xq + c0 ----
            q2_ps = psum.tile([C, NC], F32, tag="st", name="q2ps")
            for h in range(2):
                sl = slice(h * 512, (h + 1) * 512)
                nc.tensor.matmul(q2_ps[:, sl], m2T, q_raw[:, sl], start=True, stop=True)
            q2_sb = big.tile([C, NC], BF16)
            for h in range(2):
                sl = slice(h * 512, (h + 1) * 512)
                nc.vector.tensor_scalar(
                    out=q2_sb[:, sl], in0=q2_ps[:, sl], scalar1=c0_sb,
                    scalar2=None, op0=ALU.add,
                )

            # ---- main attention loop over 32 key tiles ----
            # per tile: S^T = k_tile^T @ q2 (psum) -> exp (ACT, ->sbuf bf16)
            #           h  += v^T_tile @ exp     (PSUM accumulate)
            #           db += ones    @ exp      (PSUM accumulate = denominator)
            #           acc += exp[:, 256:]      (vector f32 accumulate)
            k3 = k_raw[:].rearrange("c (t j) -> c t j", j=128)
            h_ps = psum.tile([C, NC], F32, tag="h", bufs=1)
            db_ps = psum.tile([C, DB_PE], F32, tag="b1")
            acc_sb = big.tile([128, NC - DB_PE], F32)

            def qk(t, st):
                for h in range(2):
                    sl = slice(h * 512, (h + 1) * 512)
                    nc.tensor.matmul(
                        st[:, sl], k3[:, t, :], q2_sb[:, sl], start=True, stop=True
                    )

            # upcast the first v^T chunk before the loop; the rest interleave
            # with the loop on the vector engine's slack (~8 tiles ahead).
            vt3f = vt_sb[:].rearrange("j t c -> j (t c)")
            nc.vector.tensor_copy(vt3f[:, 0 : 4 * C], vt_flat[:, 0 : 4 * C])

            # Warm AllReduce gated on an early-loop v^T upcast chunk: its
            # one-time ~11us cold CC init runs while the loop computes, so
            # the real stats AllReduce below starts on a warm CC path.
            ccw_in = dram.tile([G, 1], F32, name="ccw_in")
            ccw_out = dram.tile([G, 1], F32, name="ccw_out")
            nc.scalar.dma_start(ccw_in[:], vt_sb[0:G, 8, 0:1].bitcast(F32))
            nc.gpsimd.collective_compute(
                "AllReduce",
                ALU.add,
                replica_groups=[[0, 1, 2, 3], [4, 5, 6, 7]],
                ins=[ccw_in.opt()],
                outs=[ccw_out.opt()],
            )

            r_sb = big.tile([C, NC], BF16)
            st_tiles = {}
            st_tiles[0] = psum.tile([128, NC], F32, tag="st", name="st0")
            qk(0, st_tiles[0])
            for t in range(JT):
                if t % 4 == 0 and t + 4 < JT:
                    cs = slice((t + 4) * C, min(t + 8, JT) * C)
                    nc.vector.tensor_copy(vt3f[:, cs], vt_flat[:, cs])
                if t == 20:
                    # residual path: transfer + prep ride the late loop's
                    # DMA and DVE slack (first needed at the x epilogue)
                    nc.scalar.dma_start(vres[:], vres_in[:])
                    nc.vector.tensor_scalar(
                        out=r_sb[:], in0=vres[:], scalar1=boe_sb,
                        scalar2=None, op0=ALU.add,
                    )
                if t + 1 < JT:
                    st_tiles[t + 1] = psum.tile(
                        [128, NC], F32, tag="st", name=f"st{t + 1}"
                    )
                    qk(t + 1, st_tiles[t + 1])
                exp_t = expp.tile([128, NC], R, tag="exp")
                nc.scalar.activation(out=exp_t[:], in_=st_tiles.pop(t)[:], func=AF.Exp)
                # PV first after exp; the ones stationary load for db then
                # shadows under the second PV matmul.
                for h in range(2):
                    sl = slice(h * 512, (h + 1) * 512)
                    nc.tensor.matmul(
                        h_ps[:, sl], vt_sb[:, t, :], exp_t[:, sl],
                        start=(t == 0), stop=(t == JT - 1), skip_group_check=True,
                    )
                nc.tensor.matmul(
                    db_ps[:], ones_r[:, 0:C], exp_t[:, 0:DB_PE],
                    start=(t == 0), stop=(t == JT - 1), skip_group_check=True,
                )
                if t == 0:
                    nc.vector.tensor_copy(acc_sb[:], exp_t[:, DB_PE:NC].bitcast(F32))
                else:
                    nc.vector.tensor_add(
                        acc_sb[:], acc_sb[:], exp_t[:, DB_PE:NC].bitcast(F32)
                    )

            # ---- denominator -> 1/db broadcast across partitions ----
            # ones_f32 matmul sums acc_sb over partitions AND broadcasts.
            db2_ps = psum.tile([C, NC - DB_PE], F32, tag="st")
            half = (NC - DB_PE) // 2
            for h in range(2):
                sl = slice(h * half, (h + 1) * half)
                nc.tensor.matmul(
                    db2_ps[:, sl], ones_f, acc_sb[:, sl], start=True, stop=True
                )
            dinv_sb = big.tile([C, NC], F32)
            nc.vector.reciprocal_approx_fast(dinv_sb[:, 0:DB_PE], db_ps[:])
            nc.vector.reciprocal_approx_fast(dinv_sb[:, DB_PE:NC], db2_ps[:])

            # ---- x = W2 @ (vP * dinv) + r  (residual via identity matmul,
            # x stays in PSUM for bn_stats and the final silu); per-half
            # pipelined: half 0's matmuls and bn_stats overlap half 1's mul.
            tmp_sb = big.tile([C, NC], BF16)
            x_ps = psum.tile([C, NC], F32, tag="st", name="xps")
            bstats = big.tile([C, 2, nc.vector.BN_STATS_DIM], F32)
            for h in range(2):
                sl = slice(h * 512, (h + 1) * 512)
                nc.vector.tensor_mul(tmp_sb[:, sl], h_ps[:, sl], dinv_sb[:, sl])
                nc.tensor.matmul(
                    x_ps[:, sl], w2T, tmp_sb[:, sl],
                    start=True, stop=False, skip_group_check=True,
                )
                nc.tensor.matmul(
                    x_ps[:, sl], eye, r_sb[:, sl],
                    start=False, stop=True, skip_group_check=True,
                )

            # ---- groupnorm partial stats: per-channel [mean, E[x^2]] ----
            for hh in range(2):
                nc.vector.bn_stats(
                    out=bstats[:, hh, :], in_=x_ps[:, hh * 512 : (hh + 1) * 512]
                )
            mv = big.tile([C, nc.vector.BN_AGGR_DIM], F32)
            nc.vector.bn_aggr(out=mv[:], in_=bstats[:])
            rowstats = big.tile([C, 2], F32)
            nc.vector.tensor_copy(rowstats[:, 0:1], mv[:, 0:1])
            nc.vector.tensor_mul(rowstats[:, 1:2], mv[:, 0:1], mv[:, 0:1])
            nc.vector.tensor_add(rowstats[:, 1:2], rowstats[:, 1:2], mv[:, 1:2])
            # force the silu table load NOW so it overlaps the AllReduce
            warm_sb = const.tile([G, 1], F32)
            nc.scalar.activation(out=warm_sb[:], in_=wf[0:G, 0:1], func=AF.Silu)

            # ---- AllReduce partial stats within each batch's 4-core group ----
            gs_ps = psum.tile([G, 2], F32, tag="b1", name="gsps")
            nc.tensor.matmul(gs_ps[:], e_sb, rowstats[:], start=True, stop=True)
            gs_sb = big.tile([G, 2], F32)
            nc.vector.tensor_copy(gs_sb[:], gs_ps[:])
            cc_in = dram.tile([G, 2], F32)
            cc_out = dram.tile([G, 2], F32)
            nc.scalar.dma_start(cc_in[:], gs_sb[:])
            nc.gpsimd.collective_compute(
                "AllReduce",
                ALU.add,
                replica_groups=[[0, 1, 2, 3], [4, 5, 6, 7]],
                ins=[cc_in.opt()],
                outs=[cc_out.opt()],
            )
            own = big.tile([G, 2], F32)
            nc.sync.dma_start(own[:], cc_out[:])

            # ---- group mean / rstd -> per-channel scale+bias ----
            # own = [mean_g, E[x^2]_g] (the 1/16 group scaling is folded into
            # the host-provided e matrix).
            msr = own  # [mean, rstd] in place
            varx = big.tile([G, 1], F32)
            nc.vector.tensor_mul(varx[:], msr[:, 0:1], msr[:, 0:1])
            # var + eps = (E[x^2] + eps) - mean^2, fused
            nc.vector.scalar_tensor_tensor(
                out=varx[:], in0=msr[:, 1:2], scalar=EPS, in1=varx[:],
                op0=ALU.add, op1=ALU.subtract,
            )
            # magic-constant Newton rsqrt on [G,1] (no sqrt table). ONE
            # iteration: result is -rsqrt (sign restored via host-negated
            # gamma); |err| ~0.2% which the 2e-2 budget absorbs.
            vh = big.tile([G, 1], F32)
            nc.vector.tensor_scalar(
                out=vh[:], in0=varx[:], scalar1=0.5, scalar2=None, op0=ALU.mult
            )
            yi = big.tile([G, 1], F32)
            yint = yi[:].bitcast(I32)
            nc.vector.tensor_scalar(
                out=yint, in0=varx[:].bitcast(I32), scalar1=sh1_i, scalar2=None,
                op0=ALU.logical_shift_right,
            )
            nc.vector.tensor_sub(yint, magic_i, yint)
            t1 = big.tile([G, 1], F32)
            nc.vector.tensor_mul(t1[:], yi[:], yi[:])
            nc.vector.tensor_mul(t1[:], t1[:], vh[:])
            nc.vector.tensor_scalar(
                out=t1[:], in0=t1[:], scalar1=1.5, scalar2=None, op0=ALU.subtract
            )
            nc.vector.tensor_mul(yi[:], t1[:], yi[:])
            nc.vector.tensor_copy(msr[:, 1:2], yi[:])

            exp_ps = psum.tile([C, 2], F32, tag="b1", name="expps")
            nc.tensor.matmul(exp_ps[:], et_sb[:], msr[:], start=True, stop=True)
            mr_sb = big.tile([C, 2], F32)
            nc.vector.tensor_copy(mr_sb[:], exp_ps[:])
            fs_sb = big.tile([C, 1], F32)
            nc.vector.tensor_mul(fs_sb[:], mr_sb[:, 1:2], gamma_sb)
            fb_sb = big.tile([C, 1], F32)
            nc.vector.tensor_mul(fb_sb[:], mr_sb[:, 0:1], fs_sb[:])
            nc.vector.tensor_sub(fb_sb[:], beta_sb, fb_sb[:])

            # ---- out = silu(fs * x + fb), store overlapping the 2nd half ----
            y_sb = big.tile([C, NC], F32)
            for hh in range(2):
                sl = slice(hh * 512, (hh + 1) * 512)
                nc.scalar.activation(
                    out=y_sb[:, sl], in_=x_ps[:, sl], func=AF.Silu,
                    bias=fb_sb[:], scale=fs_sb[:],
                )
                nc.sync.dma_start(y_out[:, sl], y_sb[:, sl])

    nc.compile()
    return nc


def _get_nc():
    global _NC_CACHE
    if _NC_CACHE is None:
        _NC_CACHE = _build()
    return _NC_CACHE


def _in_maps(query, key, value, wq, bq, wk, bk, wv, bv, wo, bo, gamma, beta):
    f32 = lambda a: np.ascontiguousarray(np.asarray(a, dtype=np.float32))
    bf16 = lambda a: np.ascontiguousarray(np.asarray(a).astype(ml_dtypes.bfloat16))
    q = f32(query).reshape(B, C, N)
    k = f32(key).reshape(B, C, N)
    v = f32(value).reshape(B, C, N)
    wq, wk, wv, wo = f32(wq), f32(wk), f32(wv), f32(wo)
    bo_eff = (wo @ f32(bv).reshape(C) + f32(bo).reshape(C)).astype(np.float32)
    c0 = (wk.T @ f32(bq).reshape(C)).astype(np.float32)

    m2T = wq.T @ wk  # lhsT for q2 = (Wk^T Wq) @ xq
    w2T = (wo @ wv).T  # lhsT for x = W2 @ (vP * dinv)
    eye = np.eye(C, dtype=np.float32)
    ones = np.ones((C, C), dtype=np.float32)
    wb = bf16(np.concatenate([m2T, w2T, eye, ones], axis=1))
    e = np.zeros((C, G), dtype=np.float32)
    e[np.arange(C), np.arange(C) // (C // G)] = 1.0
    icons = np.broadcast_to(
        np.array([1, 0x5F3759DF], np.int32).view(np.float32), (C, 2)
    )
    # 1/16 group averaging folded into e; gamma negated to absorb the
    # odd-iteration sign of the on-device Newton rsqrt.
    wf = np.concatenate(
        [ones, e / 16.0,
         c0[:, None], bo_eff[:, None],
         -f32(gamma).reshape(C, 1), f32(beta).reshape(C, 1), icons], axis=1
    ).astype(np.float32)
    et = np.ascontiguousarray(e.T)  # [G, C]
    shared = {"wb": wb, "wf": np.ascontiguousarray(wf), "et": et}

    maps = []
    for p in range(NCORES):
        b, ch = divmod(p, CHUNKS)
        sl = slice(ch * NC, (ch + 1) * NC)
        # rotate the key/value token axis so this core's chunk sits at j=0;
        # attention is permutation-invariant over keys, and the residual
        # slice becomes vres at the same offset on every core.
        rot = np.roll(np.arange(N), -ch * NC)
        kr = k[b][:, rot]
        vr = v[b][:, rot]
        # v^T tiles [j, t, c]: vt[j, t*C + c] = vr[c, t*128 + j]
        vt = np.ascontiguousarray(
            vr.reshape(C, JT, 128).transpose(2, 1, 0).reshape(128, JT * C)
        )
        maps.append(
            {
                "q_in": bf16(q[b][:, sl]),
                "k_in": bf16(kr),
                "vt_in": bf16(vt),
                "vres_in": bf16(vr[:, 0:NC]),
                **shared,
            }
        )
    return maps


def kernel(query, key, value, wq, bq, wk, bk, wv, bv, wo, bo, gamma, beta):
    nc = _get_nc()
    maps = _in_maps(query, key, value, wq, bq, wk, bk, wv, bv, wo, bo, gamma, beta)
    res = run_bass_kernel_spmd(nc, maps, list(range(NCORES)))
    out = np.empty((B, C, N), dtype=np.float32)
    for p in range(NCORES):
        b, ch = divmod(p, CHUNKS)
        out[b][:, ch * NC : (ch + 1) * NC] = res.results[p]["y_out"]
    return out.reshape(B, C, 16, 16, 16)
